# revision 27
# baseline (speedup 1.0000x reference)
"""Trainium2 Bass kernel for nn_DiscreteExactLoss (joint-entropy loss).

Reference computation:
    soft_assign[b, r, :] = [1 - a[b,r], a[b,r]]          (K=2, R=10)
    joint_p[b, s]  = prod_r soft_assign[b, r, s_r]       (s in [0, 1024))
    p_a            = mean_b joint_p                       [1024]
    out            = sum_s p_a * log2(p_a)               (scalar, ~-10)

Device algorithm (per core, data-parallel over B across 8 cores):
    Accumulate MULTILINEAR MOMENTS m_T = sum_b prod_{r in T} a[b, r] for
    all 1024 subsets T. Moments factor over a 5+5 variable split:
    m_{T1 u T2} = sum_b MA[b,T1]*MC[b,T2], where MA/MC are the 32
    subset-products of each 5-var half. The 32x32 outer product summed
    over b is a TensorEngine matmul accumulated in PSUM.

    Engine split:
      - DVE: casts the first half's level-gating singleton masks
        (m = 1,2,4,8,16 hold the raw activities) and builds the remaining
        26-per-half subset products via 4 broadcast-multiply levels
        (bf16 tensor_tensor at 2x mode), with level 4 split into c-strips
        so the PE starts early.
      - ScalarE (activation tables pre-warmed by a dummy op): casts the
        second half's singletons in parallel, sized to finish before the
        PE's first matmul (ScalarE ops stall badly when their issue
        collides with the PE start / late DMA-completion sems).
      - PE: per 128-sample chunk, 4x column-tiled matmuls (tile_position
        (0, 32j)) run concurrently on the 4 col groups of the array; each
        col group accumulates its own [32,32] partial in one PSUM bank.
        Throughput is LDWEIGHTS-bound (1 weight col/cycle @ 1.2 GHz).
      - Host: sum 8 cores x 4 col-group partials, Mobius transform
        (moments -> probabilities), p*log2(p) reduction (~30k flops).

    Post-schedule optimization: all but the last matmul's semaphore
    increments are stripped (concurrent col-tiled matmuls complete in
    program order, so the final MM's single inc suffices; its waiters
    are repatched from >=128 to >=1). This removes 127 serialized
    sem-register writes from the PE stream.

    Measured 21.9-22.2us vs 25.3-25.9us baseline (device-state noise of
    up to ~20% affects individual runs). ~9.2us of that is a fixed
    runtime window cost (per-exec semaphore-clear loops + barriers
    measured between first and last "useful" instruction), ~3.3us is
    DMA-completion latency before the first byte is consumable.
"""

import math
import sys

import numpy as np

if "/opt/trn_rl_repo" not in sys.path:
    sys.path.insert(0, "/opt/trn_rl_repo")

B_FULL = 131072
R_FULL = 10
N_CORES = 8
B_LOC = B_FULL // N_CORES  # 16384
P = 128                    # SBUF partitions; samples per matmul chunk
C = B_LOC // P             # 128 sample-chunks per core (c dim)
NCH = 2                    # c-halves for pipelining
CH = C // NCH              # 64 c per half
QUAD = 4                   # chunks per col-tiled matmul quartet

_NC_CACHE = {}


def _build_module():
    if "nc" in _NC_CACHE:
        return _NC_CACHE["nc"]

    from concourse import bacc, bass, mybir, tile

    f32 = mybir.dt.float32
    bf16 = mybir.dt.bfloat16

    nc = bacc.Bacc("TRN2", target_bir_lowering=False, debug=False)

    act = nc.dram_tensor("act", [B_LOC, R_FULL], f32, kind="ExternalInput")
    msum = nc.dram_tensor("msum", [P, 32], f32, kind="ExternalOutput")

    # dram view [p, c, r]: sample b = p*C + c
    act_pcr = act.ap().rearrange("(p c) r -> p c r", p=P)

    with tile.TileContext(nc) as tc:
        with (
            tc.tile_pool(name="a0", bufs=1) as a0_pool,
            tc.tile_pool(name="mac", bufs=1) as mac_pool,
            tc.tile_pool(name="outp", bufs=1) as out_pool,
            tc.tile_pool(name="psum", bufs=1, space=bass.MemorySpace.PSUM) as psum_pool,
        ):
            # raw activity, f32, natural layout [p, c, r]
            a0 = a0_pool.tile([P, C, R_FULL], f32)
            # mask table: mac[p, h, m, c] = prod of half-h vars in mask m
            mac = mac_pool.tile([P, 2, 32, C], bf16)
            warm = mac_pool.tile([P, 1], bf16)
            psum_acc = psum_pool.tile([P, 32], f32)

            # ---- DMAs: two 320KB chunks (HWDGE, sequential ring).
            # First lands ~10.0us, second ~10.8us after window start; a
            # finer split delays the LAST chunk's completion sem, which
            # stalls whoever consumes it (measured +1.5-2.5us).
            for ch in range(NCH):
                cs = slice(ch * CH, (ch + 1) * CH)
                nc.sync.dma_start(out=a0[:, cs, :], in_=act_pcr[:, cs, :])

            # Warm the ScalarE activation tables immediately: the first
            # ACTIVATE pays ~2.7us of ACT_TABLE_LOAD + drain, absorbed
            # here while the DMAs are still in flight.
            nc.scalar.copy(warm[:, :], warm[:, :])

            # m=0 column := 1.0 (empty product); used only by the matmul.
            nc.gpsimd.memset(mac[:, :, 0:1, :], 1.0)

            QGRP = C // QUAD  # 32 quartets in the whole accumulation

            def emit_pe(c_lo, c_hi):
                for q in range(c_lo // QUAD, c_hi // QUAD):
                    for jj in range(QUAD):
                        c = q * QUAD + jj
                        nc.tensor.matmul(
                            psum_acc[32 * jj:32 * jj + 32, :],
                            mac[:, 0, :, c],   # lhsT [K=128, M=32]
                            mac[:, 1, :, c],   # rhs  [K=128, N=32]
                            start=(q == 0),
                            stop=(q == QGRP - 1),
                            tile_position=(0, 32 * jj),
                        )

            # --- schedule (half-granular; the measured-fastest layout) ---
            # Half 0 (c 0-63): DVE casts the level-gating singletons
            # itself; ScalarE (warmed) does {16} plus all of half 1's
            # singletons, finishing just before the PE's first matmul.
            for ch in range(NCH):
                cs = slice(ch * CH, (ch + 1) * CH)
                a_h = a0[:, cs, :].rearrange("p c (h l) -> p h l c", h=2)
                if ch == 0:
                    nc.vector.tensor_copy(mac[:, :, 1:3, cs], a_h[:, :, 0:2, :])
                    nc.vector.tensor_copy(mac[:, :, 4:12:4, cs], a_h[:, :, 2:4, :])
                    nc.scalar.copy(mac[:, :, 16:17, cs], a_h[:, :, 4:5, :])
                else:
                    nc.scalar.copy(mac[:, :, 1:3, cs], a_h[:, :, 0:2, :])
                    nc.scalar.copy(mac[:, :, 4:12:4, cs], a_h[:, :, 2:4, :])
                    nc.scalar.copy(mac[:, :, 16:17, cs], a_h[:, :, 4:5, :])

                # DVE broadcast-multiply cascade, levels 1-3 over the half
                for lvl in range(1, 4):
                    j = 1 << lvl
                    a_bc = mac[:, :, j:j + 1, cs].broadcast_to([P, 2, j - 1, CH])
                    nc.vector.tensor_tensor(
                        mac[:, :, j + 1:2 * j, cs],
                        mac[:, :, 1:j, cs],
                        a_bc,
                        mybir.AluOpType.mult,
                    )
                # level 4 in strips feeding the PE; final strips smaller so
                # the PE's last dependency closes sooner
                strips = [(0, 32), (32, 64)] if ch == 0 else \
                         [(64, 96), (96, 112), (112, 128)]
                for (s_lo, s_hi) in strips:
                    ss = slice(s_lo, s_hi)
                    a_bc = mac[:, :, 16:17, ss].broadcast_to(
                        [P, 2, 15, s_hi - s_lo])
                    nc.vector.tensor_tensor(
                        mac[:, :, 17:32, ss],
                        mac[:, :, 1:16, ss],
                        a_bc,
                        mybir.AluOpType.mult,
                    )
                    emit_pe(s_lo, s_hi)

            out_sb = out_pool.tile([P, 32], f32)
            nc.vector.tensor_copy(out_sb[:, :], psum_acc[:, :])
            nc.sync.dma_start(out=msum[:, :], in_=out_sb[:, :])

    # Thin the Tile-generated counting-semaphore traffic: instructions
    # on one engine complete in program order (and concurrent col-tiled
    # matmuls do too, HW-verified), so for each counting sem only the
    # increments that satisfy an actual waiter threshold are needed.
    # Keep exactly those and renumber every waiter to its threshold's
    # rank among the kept incs (walrus requires inc values of 1).
    # Removes 127 of 128 matmul incs (serialized sem-register writes in
    # a ~37ns/instruction stream) and the non-threshold DVE/ScalarE
    # chain incs (lets consecutive DVE ops issue back-to-back).
    all_ins = [
        ins
        for func in nc.m.functions
        for block in func.blocks
        for ins in block.instructions
    ]
    updaters = {}   # sem id -> [instruction] in program order
    waits = {}      # sem id -> [SyncWait]
    disqualified = set()
    for ins in all_ins:
        si = ins.sync_info
        if si is None:
            continue
        for u in si.on_update:
            if u.sync_type != "semaphore":
                continue
            if u.update_mode != "sem-inc" or u.update_value != 1:
                disqualified.add(u.id)
                continue
            updaters.setdefault(u.id, []).append(ins)
        for w in si.on_wait:
            if w.sync_type != "semaphore":
                continue
            if w.wait_mode != "sem-ge-imm":
                disqualified.add(w.id)
                continue
            waits.setdefault(w.id, []).append(w)

    for sem, ups in updaters.items():
        if sem in disqualified or sem not in waits:
            continue
        if len({ins.engine for ins in ups}) != 1:
            continue  # cross-engine counting: order not guaranteed
        thresholds = sorted({w.wait_value for w in waits[sem]})
        if not thresholds or thresholds[-1] > len(ups) or thresholds[0] < 1:
            continue
        keep = set(thresholds)  # 1-based updater indices to keep
        rank = {v: i + 1 for i, v in enumerate(thresholds)}
        for idx, ins in enumerate(ups, start=1):
            if idx not in keep:
                ins.sync_info.on_update = [
                    u for u in ins.sync_info.on_update
                    if not (u.sync_type == "semaphore" and u.id == sem)
                ]
        for w in waits[sem]:
            w.wait_value = rank[w.wait_value]

    # Bacc modules carry virtual registers until compile() runs; the
    # bass2jax/PJRT path serializes nc as-is, so allocate them now.
    nc.compile()
    _NC_CACHE["nc"] = nc
    return nc


def _ensure_ntff_hook():
    """The agent image's antenv package lacks axon_hooks; synthesize it so
    run_bass_kernel_spmd(trace=True) can find the NTFF profile hook."""
    import types

    try:
        from antenv.axon_hooks import get_axon_ntff_profile_hook  # noqa: F401
        return
    except ImportError:
        pass
    import antenv

    mod = types.ModuleType("antenv.axon_hooks")
    state = {"hook": None}
    mod.set_axon_ntff_profile_hook = lambda h: state.__setitem__("hook", h)
    mod.get_axon_ntff_profile_hook = lambda: state["hook"]
    antenv.axon_hooks = mod
    sys.modules["antenv.axon_hooks"] = mod

    try:
        from trn_agent_boot.trn_boot import _ntff_profile_via_ctypes

        hook = _ntff_profile_via_ctypes("/opt/axon/libaxon_pjrt.so")
        if hook is not None:
            mod.set_axon_ntff_profile_hook(hook)
    except Exception:
        pass


def _run_on_device(activity, trace=False):
    from concourse.bass_utils import run_bass_kernel_spmd

    if trace:
        _ensure_ntff_hook()
    nc = _build_module()
    shards = np.ascontiguousarray(activity.astype(np.float32)).reshape(
        N_CORES, B_LOC, R_FULL
    )
    in_maps = [{"act": np.ascontiguousarray(shards[i])} for i in range(N_CORES)]
    res = run_bass_kernel_spmd(
        nc, in_maps, core_ids=list(range(N_CORES)), trace=trace
    )
    return res


def _finish_on_host(per_core_msums):
    # total moment sums over all B samples; fold the 4 col-group partials
    msum = np.zeros((32, 32), dtype=np.float64)
    for part in per_core_msums:
        p128 = part.astype(np.float64).reshape(4, 32, 32)
        msum += p128.sum(axis=0)
    m = (msum / B_FULL).reshape(-1)  # [1024] mean moments

    # Mobius transform per bit: p(bit=0) = m(without) - m(with)
    p = m.copy()
    idx = np.arange(1024)
    for bit in range(10):
        step = 1 << bit
        lo = idx[(idx & step) == 0]
        p[lo] = p[lo] - p[lo | step]

    p = p.astype(np.float32)
    p_safe = np.clip(p, 1e-12, None)
    log_k_p = np.log(p_safe) / math.log(2.0)
    joint_h = -np.sum(p * log_k_p)
    return np.array(-joint_h, dtype=np.float32)


def kernel(activity):
    res = _run_on_device(activity, trace=False)
    return _finish_on_host([r["msum"] for r in res.results])


def kernel_profiled(activity):
    """Like kernel() but with NTFF tracing; returns (output, exec_time_ns)."""
    res = _run_on_device(activity, trace=True)
    out = _finish_on_host([r["msum"] for r in res.results])
    return out, res.exec_time_ns


# revision 28
# speedup vs baseline: 1.0225x; 1.0225x over previous
"""Trainium2 Bass kernel for nn_DiscreteExactLoss (joint-entropy loss).

Reference computation:
    soft_assign[b, r, :] = [1 - a[b,r], a[b,r]]          (K=2, R=10)
    joint_p[b, s]  = prod_r soft_assign[b, r, s_r]       (s in [0, 1024))
    p_a            = mean_b joint_p                       [1024]
    out            = sum_s p_a * log2(p_a)               (scalar, ~-10)

Device algorithm (per core, data-parallel over B across 8 cores):
    Accumulate MULTILINEAR MOMENTS m_T = sum_b prod_{r in T} a[b, r] for
    all 1024 subsets T. Moments factor over a 5+5 variable split:
    m_{T1 u T2} = sum_b MA[b,T1]*MC[b,T2], where MA/MC are the 32
    subset-products of each 5-var half. The 32x32 outer product summed
    over b is a TensorEngine matmul accumulated in PSUM.

    Engine split:
      - DVE: casts the first half's level-gating singleton masks
        (m = 1,2,4,8,16 hold the raw activities) and builds the remaining
        26-per-half subset products via 4 broadcast-multiply levels
        (bf16 tensor_tensor at 2x mode), with level 4 split into c-strips
        so the PE starts early.
      - ScalarE (activation tables pre-warmed by a dummy op): casts the
        second half's singletons in parallel, sized to finish before the
        PE's first matmul (ScalarE ops stall badly when their issue
        collides with the PE start / late DMA-completion sems).
      - PE: per 128-sample chunk, 4x column-tiled matmuls (tile_position
        (0, 32j)) run concurrently on the 4 col groups of the array; each
        col group accumulates its own [32,32] partial in one PSUM bank.
        Throughput is LDWEIGHTS-bound (1 weight col/cycle @ 1.2 GHz).
      - Host: sum 8 cores x 4 col-group partials, Mobius transform
        (moments -> probabilities), p*log2(p) reduction (~30k flops).

    Post-schedule optimization: all but the last matmul's semaphore
    increments are stripped (concurrent col-tiled matmuls complete in
    program order, so the final MM's single inc suffices; its waiters
    are repatched from >=128 to >=1). This removes 127 serialized
    sem-register writes from the PE stream.

    Measured 21.9-22.2us vs 25.3-25.9us baseline (device-state noise of
    up to ~20% affects individual runs). ~9.2us of that is a fixed
    runtime window cost (per-exec semaphore-clear loops + barriers
    measured between first and last "useful" instruction), ~3.3us is
    DMA-completion latency before the first byte is consumable.
"""

import math
import sys

import numpy as np

if "/opt/trn_rl_repo" not in sys.path:
    sys.path.insert(0, "/opt/trn_rl_repo")

B_FULL = 131072
R_FULL = 10
N_CORES = 8
B_LOC = B_FULL // N_CORES  # 16384
P = 128                    # SBUF partitions; samples per matmul chunk
C = B_LOC // P             # 128 sample-chunks per core (c dim)
NCH = 2                    # c-halves for pipelining
CH = C // NCH              # 64 c per half
QUAD = 4                   # chunks per col-tiled matmul quartet

_NC_CACHE = {}


def _build_module():
    if "nc" in _NC_CACHE:
        return _NC_CACHE["nc"]

    from concourse import bacc, bass, mybir, tile

    f32 = mybir.dt.float32
    bf16 = mybir.dt.bfloat16

    nc = bacc.Bacc("TRN2", target_bir_lowering=False, debug=False)

    act = nc.dram_tensor("act", [B_LOC, R_FULL], f32, kind="ExternalInput")
    msum = nc.dram_tensor("msum", [P, 32], f32, kind="ExternalOutput")

    # dram view [p, c, r]: sample b = p*C + c
    act_pcr = act.ap().rearrange("(p c) r -> p c r", p=P)

    with tile.TileContext(nc) as tc:
        with (
            tc.tile_pool(name="a0", bufs=1) as a0_pool,
            tc.tile_pool(name="mac", bufs=1) as mac_pool,
            tc.tile_pool(name="outp", bufs=1) as out_pool,
            tc.tile_pool(name="psum", bufs=1, space=bass.MemorySpace.PSUM) as psum_pool,
        ):
            # raw activity, f32, natural layout [p, c, r]
            a0 = a0_pool.tile([P, C, R_FULL], f32)
            # mask table: mac[p, h, m, c] = prod of half-h vars in mask m
            mac = mac_pool.tile([P, 2, 32, C], bf16)
            warm = mac_pool.tile([P, 1], bf16)
            psum_acc = psum_pool.tile([P, 32], f32)

            # ---- DMAs: two 320KB chunks (HWDGE, sequential ring).
            # First lands ~10.0us, second ~10.8us after window start; a
            # finer split delays the LAST chunk's completion sem, which
            # stalls whoever consumes it (measured +1.5-2.5us).
            for ch in range(NCH):
                cs = slice(ch * CH, (ch + 1) * CH)
                nc.sync.dma_start(out=a0[:, cs, :], in_=act_pcr[:, cs, :])

            # Warm the ScalarE activation tables immediately: the first
            # ACTIVATE pays ~2.7us of ACT_TABLE_LOAD + drain, absorbed
            # here while the DMAs are still in flight.
            nc.scalar.copy(warm[:, :], warm[:, :])

            # m=0 column := 1.0 (empty product); used only by the matmul.
            nc.gpsimd.memset(mac[:, :, 0:1, :], 1.0)

            QGRP = C // QUAD  # 32 quartets in the whole accumulation

            def emit_pe(c_lo, c_hi):
                for q in range(c_lo // QUAD, c_hi // QUAD):
                    for jj in range(QUAD):
                        c = q * QUAD + jj
                        nc.tensor.matmul(
                            psum_acc[32 * jj:32 * jj + 32, :],
                            mac[:, 0, :, c],   # lhsT [K=128, M=32]
                            mac[:, 1, :, c],   # rhs  [K=128, N=32]
                            start=(q == 0),
                            stop=(q == QGRP - 1),
                            tile_position=(0, 32 * jj),
                        )

            # --- schedule (half-granular; the measured-fastest layout) ---
            # Half 0 (c 0-63): DVE casts the level-gating singletons
            # itself; ScalarE (warmed) does {16} plus all of half 1's
            # singletons, finishing just before the PE's first matmul.
            for ch in range(NCH):
                cs = slice(ch * CH, (ch + 1) * CH)
                a_h = a0[:, cs, :].rearrange("p c (h l) -> p h l c", h=2)
                if ch == 0:
                    nc.vector.tensor_copy(mac[:, :, 1:3, cs], a_h[:, :, 0:2, :])
                    nc.vector.tensor_copy(mac[:, :, 4:12:4, cs], a_h[:, :, 2:4, :])
                    nc.scalar.copy(mac[:, :, 16:17, cs], a_h[:, :, 4:5, :])
                else:
                    nc.scalar.copy(mac[:, :, 1:3, cs], a_h[:, :, 0:2, :])
                    nc.scalar.copy(mac[:, :, 4:12:4, cs], a_h[:, :, 2:4, :])
                    nc.scalar.copy(mac[:, :, 16:17, cs], a_h[:, :, 4:5, :])

                # DVE broadcast-multiply cascade, levels 1-3 over the half
                for lvl in range(1, 4):
                    j = 1 << lvl
                    a_bc = mac[:, :, j:j + 1, cs].broadcast_to([P, 2, j - 1, CH])
                    nc.vector.tensor_tensor(
                        mac[:, :, j + 1:2 * j, cs],
                        mac[:, :, 1:j, cs],
                        a_bc,
                        mybir.AluOpType.mult,
                    )
                # level 4 in strips feeding the PE; final strips smaller so
                # the PE's last dependency closes sooner
                strips = [(0, 32), (32, 64)] if ch == 0 else \
                         [(64, 96), (96, 112), (112, 128)]
                for (s_lo, s_hi) in strips:
                    ss = slice(s_lo, s_hi)
                    a_bc = mac[:, :, 16:17, ss].broadcast_to(
                        [P, 2, 15, s_hi - s_lo])
                    nc.vector.tensor_tensor(
                        mac[:, :, 17:32, ss],
                        mac[:, :, 1:16, ss],
                        a_bc,
                        mybir.AluOpType.mult,
                    )
                    emit_pe(s_lo, s_hi)

            out_sb = out_pool.tile([P, 32], f32)
            nc.vector.tensor_copy(out_sb[:, :], psum_acc[:, :])
            nc.sync.dma_start(out=msum[:, :], in_=out_sb[:, :])

    # Strip the redundant per-matmul semaphore increments: concurrent
    # col-tiled matmuls complete in program order (HW-verified; see the
    # tensor-engine docs), so only the final matmul needs to signal —
    # with the full count, keeping every >=128 waiter valid. The 127
    # removed sem-register writes otherwise serialize against the
    # LDWEIGHTS/MATMUL stream (~26ns each).
    mms = [
        ins
        for func in nc.m.functions
        for block in func.blocks
        for ins in block.instructions
        if ins.opcode == "Matmult"
    ]
    sem_ids = set()
    for ins in mms:
        for u in ins.sync_info.on_update:
            assert u.update_mode == "sem-inc" and u.update_value == 1
            sem_ids.add(u.id)
    assert len(sem_ids) == 1, sem_ids
    mm_sem = sem_ids.pop()
    for ins in mms[:-1]:
        ins.sync_info.on_update = []
    # walrus asserts UpdateValue == 1, so the final matmul incs by 1 and
    # every waiter drops from >=128 to >=1.
    n_waiters = 0
    for func in nc.m.functions:
        for block in func.blocks:
            for ins in block.instructions:
                si = ins.sync_info
                if si is None:
                    continue
                for w in si.on_wait:
                    if w.id == mm_sem:
                        assert w.wait_mode == "sem-ge-imm"
                        assert w.wait_value == len(mms)
                        w.wait_value = 1
                        n_waiters += 1
    assert n_waiters >= 1, "no sem waiters found to patch"

    # Bacc modules carry virtual registers until compile() runs; the
    # bass2jax/PJRT path serializes nc as-is, so allocate them now.
    nc.compile()
    _NC_CACHE["nc"] = nc
    return nc


def _ensure_ntff_hook():
    """The agent image's antenv package lacks axon_hooks; synthesize it so
    run_bass_kernel_spmd(trace=True) can find the NTFF profile hook."""
    import types

    try:
        from antenv.axon_hooks import get_axon_ntff_profile_hook  # noqa: F401
        return
    except ImportError:
        pass
    import antenv

    mod = types.ModuleType("antenv.axon_hooks")
    state = {"hook": None}
    mod.set_axon_ntff_profile_hook = lambda h: state.__setitem__("hook", h)
    mod.get_axon_ntff_profile_hook = lambda: state["hook"]
    antenv.axon_hooks = mod
    sys.modules["antenv.axon_hooks"] = mod

    try:
        from trn_agent_boot.trn_boot import _ntff_profile_via_ctypes

        hook = _ntff_profile_via_ctypes("/opt/axon/libaxon_pjrt.so")
        if hook is not None:
            mod.set_axon_ntff_profile_hook(hook)
    except Exception:
        pass


def _run_on_device(activity, trace=False):
    from concourse.bass_utils import run_bass_kernel_spmd

    if trace:
        _ensure_ntff_hook()
    nc = _build_module()
    shards = np.ascontiguousarray(activity.astype(np.float32)).reshape(
        N_CORES, B_LOC, R_FULL
    )
    in_maps = [{"act": np.ascontiguousarray(shards[i])} for i in range(N_CORES)]
    res = run_bass_kernel_spmd(
        nc, in_maps, core_ids=list(range(N_CORES)), trace=trace
    )
    return res


def _finish_on_host(per_core_msums):
    # total moment sums over all B samples; fold the 4 col-group partials
    msum = np.zeros((32, 32), dtype=np.float64)
    for part in per_core_msums:
        p128 = part.astype(np.float64).reshape(4, 32, 32)
        msum += p128.sum(axis=0)
    m = (msum / B_FULL).reshape(-1)  # [1024] mean moments

    # Mobius transform per bit: p(bit=0) = m(without) - m(with)
    p = m.copy()
    idx = np.arange(1024)
    for bit in range(10):
        step = 1 << bit
        lo = idx[(idx & step) == 0]
        p[lo] = p[lo] - p[lo | step]

    p = p.astype(np.float32)
    p_safe = np.clip(p, 1e-12, None)
    log_k_p = np.log(p_safe) / math.log(2.0)
    joint_h = -np.sum(p * log_k_p)
    return np.array(-joint_h, dtype=np.float32)


def kernel(activity):
    res = _run_on_device(activity, trace=False)
    return _finish_on_host([r["msum"] for r in res.results])


def kernel_profiled(activity):
    """Like kernel() but with NTFF tracing; returns (output, exec_time_ns)."""
    res = _run_on_device(activity, trace=True)
    out = _finish_on_host([r["msum"] for r in res.results])
    return out, res.exec_time_ns
